# revision 11
# baseline (speedup 1.0000x reference)
"""Trainium2 Bass kernel for CTM sampling (nn_CTM_30846455120449).

Reference computation (bow is unused by the output):
    theta = softmax(alpha)                    # [K]
    B     = softmax(beta, axis=1)             # [K, K]
    L     = cholesky(sigma)                   # [K, K]
    z     = mu + eps @ L.T                    # [N, K]
    eta   = softmax(z @ B, axis=1)            # [N, K]
    gamma = eta * theta + RHO; gamma /= gamma.sum(1, keepdims=True)

Strategy:
  * All [K,K]-sized math folds on host:  C = L.T @ B,  c0 = mu@B + log(theta).
    Then logits' l'_ij = (eps @ C)_ij + c0_j  (theta folded into the bias), and
    with e' = exp(l'), T_i = sum_j e'_ij, V_i = sum_j e'_ij/theta_j:
        gamma_ij = (e'_ij + RHO*V_i) / (T_i + K*RHO*V_i)
    For uniform theta (alpha = const), V = K*T exactly, so
        gamma_ij = e'_ij * s1_i + CONST,  s1_i = 1/(CDEN*T_i),
        CDEN = 1 + K^2*RHO,  CONST = K*RHO/CDEN.
  * Device streams eps row-tiles [128, 512]: 4 bf16 matmuls (pre-transposed eps
    chunks stationary, C chunks moving) + a 3-row bias matmul into PSUM, one
    ScalarE Exp with accum_out (gives e' and T), tiny VectorE ops + one
    tensor_scalar for gamma, DMA out.  Data-parallel over 8 cores (16384 rows
    per core).
  * eps is pre-transposed/cast to bf16 on host so no on-device transposes are
    needed; the bias stays fp32-exact via a 3-row bf16 hi/mid/lo split.
"""

import numpy as np
import ml_dtypes

_N = 131072
_K = 512
_RHO = 0.01
_NCORES = 8
_P = 128
_KC = _K // _P          # 4 contraction chunks of 128
_NSHARD = _N // _NCORES  # 16384 rows per core
_NTILES = _NSHARD // _P  # 128 tiles per core

_BF16 = ml_dtypes.bfloat16

_prog_cache = {}
_trace = False        # set True externally to profile the run
_last_results = None  # BassKernelResults of the most recent run


_G = 8            # row-tiles per DMA group (G*128 rows, >=1MiB per transfer)
_FP8 = True       # use fp8e4 + DoubleRow matmuls instead of bf16


def _build_program(ntiles, general_theta, fp8):
    import concourse.bass as bass
    import concourse.tile as tile
    from concourse import bacc, mybir

    f32 = mybir.dt.float32
    bf16 = mybir.dt.bfloat16
    fp8e4 = mybir.dt.float8e4
    edt = fp8e4 if fp8 else bf16
    AF = mybir.ActivationFunctionType
    OP = mybir.AluOpType
    DR = mybir.MatmulPerfMode.DoubleRow
    nshard = ntiles * _P
    G = _G
    ng = ntiles // G
    assert ntiles % G == 0

    CDEN = 1.0 + _K * _K * _RHO
    CONST = (_K * _RHO) / CDEN

    nc = bacc.Bacc("TRN2", target_bir_lowering=False, debug=False)
    epsT_d = nc.declare_dram_parameter("epsT", [ng, _P, G, _KC, _P], edt, isOutput=False)
    C_d = nc.declare_dram_parameter("Cmat", [_P, _KC, _K], edt, isOutput=False)
    c0_d = nc.declare_dram_parameter("c0rows", [3, _K], bf16, isOutput=False)
    if general_theta:
        invth_d = nc.declare_dram_parameter("invtheta", [_P, _K], f32, isOutput=False)
    gamma_d = nc.declare_dram_parameter("gamma", [nshard, _K], f32, isOutput=True)
    # [ng, P(doc-in-tile), G(tile), K] view of the row-major output
    gv = gamma_d[:].rearrange("(ng t d) k -> ng d t k", t=G, d=_P)

    with tile.TileContext(nc) as tc:
        with (
            tc.tile_pool(name="const", bufs=1) as constp,
            tc.tile_pool(name="eps", bufs=4) as epsp,
            tc.tile_pool(name="psum", bufs=4, space=bass.MemorySpace.PSUM) as psump,
            tc.tile_pool(name="work", bufs=2 * _G) as workp,
            tc.tile_pool(name="gout", bufs=3) as goutp,
            tc.tile_pool(name="stat", bufs=3) as statp,
        ):
            Ct = constp.tile([_P, _KC, _K], edt)
            nc.gpsimd.dma_start(Ct[:], C_d[:])
            c0t = constp.tile([3, _K], bf16)
            nc.gpsimd.dma_start(c0t[:], c0_d[:])
            ones3 = constp.tile([3, _P], bf16)
            nc.vector.memset(ones3[:], 1.0)
            if general_theta:
                invtht = constp.tile([_P, _K], f32)
                nc.gpsimd.dma_start(invtht[:], invth_d[:])

            inv_scale = 1.0  # set per build for fp8 via module global below
            if fp8:
                inv_scale = _fp8_inv_scale[0]

            for gi in range(ng):
                egt = epsp.tile([_P, G, _KC, _P], edt, tag="eps")
                nc.gpsimd.dma_start(egt[:], epsT_d[gi])
                gbuf = goutp.tile([_P, G, _K], f32, tag="gbuf")

                es = []
                Tg = statp.tile([_P, G], f32, tag="T")
                Vg = statp.tile([_P, G], f32, tag="V") if general_theta else None
                for t in range(G):
                    ps = psump.tile([_P, _K], f32, tag="ps")
                    # bias first: resets PSUM to the sum of 3 bias rows
                    nc.tensor.matmul(ps[:], ones3[:], c0t[:], start=True, stop=False)
                    if fp8:
                        for c in (0, 2):
                            nc.tensor.matmul(
                                ps[:], egt[:, t, c:c + 2, :], Ct[:, c:c + 2, :],
                                start=False, stop=(c == 2), perf_mode=DR,
                            )
                    else:
                        for c in range(_KC):
                            nc.tensor.matmul(
                                ps[:], egt[:, t, c, :], Ct[:, c, :],
                                start=False, stop=(c == _KC - 1),
                            )

                    e = workp.tile([_P, _K], f32, tag="e")
                    nc.scalar.activation(e[:], ps[:], AF.Exp,
                                         scale=inv_scale,
                                         accum_out=Tg[:, t:t + 1])
                    es.append(e)
                    if general_theta:
                        scratch = workp.tile([_P, _K], f32, tag="scratch")
                        nc.vector.tensor_tensor_reduce(
                            out=scratch[:], in0=e[:], in1=invtht[:],
                            scale=1.0, scalar=0.0,
                            op0=OP.mult, op1=OP.add,
                            accum_out=Vg[:, t:t + 1],
                        )

                # group-batched stats: one mul + one reciprocal per 8 tiles
                s1g = statp.tile([_P, G], f32, tag="s1")
                if not general_theta:
                    dtmp = statp.tile([_P, G], f32, tag="dtmp")
                    nc.vector.tensor_scalar_mul(dtmp[:], Tg[:], CDEN)
                    nc.vector.reciprocal(s1g[:], dtmp[:])
                    for t in range(G):
                        nc.vector.tensor_scalar(
                            gbuf[:, t, :], es[t][:], s1g[:, t:t + 1], CONST,
                            OP.mult, OP.add,
                        )
                else:
                    # D = K*RHO*V + T;  s1 = 1/D;  s0 = RHO*V*s1
                    Dg = statp.tile([_P, G], f32, tag="D")
                    nc.vector.tensor_scalar_mul(Dg[:], Vg[:], float(_K * _RHO))
                    nc.vector.tensor_add(Dg[:], Dg[:], Tg[:])
                    nc.vector.reciprocal(s1g[:], Dg[:])
                    s0g = statp.tile([_P, G], f32, tag="s0")
                    nc.vector.tensor_mul(s0g[:], Vg[:], s1g[:])
                    nc.vector.tensor_scalar_mul(s0g[:], s0g[:], _RHO)
                    for t in range(G):
                        nc.vector.tensor_scalar(
                            gbuf[:, t, :], es[t][:], s1g[:, t:t + 1],
                            s0g[:, t:t + 1], OP.mult, OP.add,
                        )

                nc.gpsimd.dma_start(gv[gi], gbuf[:])
    nc.compile()
    return nc


_fp8_inv_scale = [1.0]  # exp-input rescale for the fp8 build (2**-s)


def _softmax_rows(x):
    m = x.max(axis=-1, keepdims=True)
    e = np.exp(x - m)
    return e / e.sum(axis=-1, keepdims=True)


def _host_prep(alpha, beta, sigma, mu, eps):
    """Fold the small parameters; shard + transpose/cast eps."""
    theta = _softmax_rows(alpha.astype(np.float64))            # [K]
    B = _softmax_rows(beta.astype(np.float64))                 # [K, K]
    L = np.linalg.cholesky(sigma.astype(np.float64))           # [K, K]
    C = L.T @ B                                                # [K, K]
    c0 = mu.astype(np.float64) @ B + np.log(theta)             # [K]

    uniform = bool(np.max(np.abs(theta - 1.0 / _K)) < 1e-12)

    # fp8 pre-scale: put max|C|*2^s around ~96 (TRN e4m3 max normal is 240)
    scale_log2 = 0
    if _FP8:
        maxc = float(np.abs(C).max())
        scale_log2 = int(np.floor(np.log2(96.0 / maxc))) if maxc > 0 else 0
    scl = float(2.0 ** scale_log2)

    # C chunk layout [P, KC, K]: element [p, c, j] = C[c*P + p, j]
    Cb = np.ascontiguousarray(
        (C * scl).reshape(_KC, _P, _K).transpose(1, 0, 2)
    ).astype(_EDT())

    # 3-row hi/mid/lo bf16 split of c0*scl so the bias is fp32-exact on device
    c0f = (c0 * scl).astype(np.float32)
    r0 = c0f.astype(_BF16)
    r1 = (c0f - r0.astype(np.float32)).astype(_BF16)
    r2 = (c0f - r0.astype(np.float32) - r1.astype(np.float32)).astype(_BF16)
    c0rows = np.stack([r0, r1, r2], axis=0)                    # [3, K] bf16

    invtheta = None
    if not uniform:
        invtheta = np.broadcast_to(
            (1.0 / theta).astype(np.float32), (_P, _K)
        ).copy()

    shards = [
        _prep_eps_shard(eps[core * _NSHARD:(core + 1) * _NSHARD])
        for core in range(_NCORES)
    ]
    return Cb, c0rows, invtheta, uniform, scale_log2, shards


def _EDT():
    return ml_dtypes.float8_e4m3 if _FP8 else _BF16


def _prep_eps_shard(sh):
    """[rows, K] -> [ng, P(k-sub), G(tile), KC, P(doc)] grouped-transposed."""
    ntiles = sh.shape[0] // _P
    ng = ntiles // _G
    sh5 = sh.reshape(ng, _G, _P, _KC, _P)                 # [g, t, d, c, p]
    return np.ascontiguousarray(sh5.transpose(0, 4, 1, 3, 2)).astype(_EDT())


def kernel(bow, alpha, beta, sigma, mu, eps):
    from concourse.bass_utils import run_bass_kernel_spmd

    Cb, c0rows, invtheta, uniform, scale_log2, shards = _host_prep(
        alpha, beta, sigma, mu, eps)

    key = (_NTILES, not uniform, _FP8, scale_log2)
    if key not in _prog_cache:
        _fp8_inv_scale[0] = float(2.0 ** -scale_log2)
        _prog_cache[key] = _build_program(_NTILES, not uniform, _FP8)
    nc = _prog_cache[key]

    in_maps = []
    for core in range(_NCORES):
        m = {"epsT": shards[core], "Cmat": Cb, "c0rows": c0rows}
        if not uniform:
            m["invtheta"] = invtheta
        in_maps.append(m)

    global _last_results
    res = run_bass_kernel_spmd(nc, in_maps, list(range(_NCORES)), trace=_trace)
    _last_results = res
    out = np.concatenate([res.results[i]["gamma"] for i in range(_NCORES)], axis=0)
    return np.ascontiguousarray(out.astype(np.float32))


# revision 12
# speedup vs baseline: 1.4597x; 1.4597x over previous
"""Trainium2 Bass kernel for CTM sampling (nn_CTM_30846455120449).

Reference computation (bow is unused by the output):
    theta = softmax(alpha)                    # [K]
    B     = softmax(beta, axis=1)             # [K, K]
    L     = cholesky(sigma)                   # [K, K]
    z     = mu + eps @ L.T                    # [N, K]
    eta   = softmax(z @ B, axis=1)            # [N, K]
    gamma = eta * theta + RHO; gamma /= gamma.sum(1, keepdims=True)

Strategy:
  * All [K,K]-sized math folds on host:  C = L.T @ B,  c0 = mu@B + log(theta).
    Then logits' l'_ij = (eps @ C)_ij + c0_j  (theta folded into the bias), and
    with e' = exp(l'), T_i = sum_j e'_ij, V_i = sum_j e'_ij/theta_j:
        gamma_ij = (e'_ij + RHO*V_i) / (T_i + K*RHO*V_i)
    For uniform theta (alpha = const), V = K*T exactly, so
        gamma_ij = e'_ij * s1_i + CONST,  s1_i = 1/(CDEN*T_i),
        CDEN = 1 + K^2*RHO,  CONST = K*RHO/CDEN.
  * Device streams eps row-tiles [128, 512]: 4 bf16 matmuls (pre-transposed eps
    chunks stationary, C chunks moving) + a 3-row bias matmul into PSUM, one
    ScalarE Exp with accum_out (gives e' and T), tiny VectorE ops + one
    tensor_scalar for gamma, DMA out.  Data-parallel over 8 cores (16384 rows
    per core).
  * eps is pre-transposed/cast to bf16 on host so no on-device transposes are
    needed; the bias stays fp32-exact via a 3-row bf16 hi/mid/lo split.
"""

import numpy as np
import ml_dtypes

_N = 131072
_K = 512
_RHO = 0.01
_NCORES = 8
_P = 128
_KC = _K // _P          # 4 contraction chunks of 128
_NSHARD = _N // _NCORES  # 16384 rows per core
_NTILES = _NSHARD // _P  # 128 tiles per core

_BF16 = ml_dtypes.bfloat16

_prog_cache = {}
_trace = False        # set True externally to profile the run
_last_results = None  # BassKernelResults of the most recent run


_G = 8            # row-tiles per DMA group (G*128 rows, >=1MiB per transfer)
_FP8 = True       # use fp8e4 + DoubleRow matmuls instead of bf16


def _build_program(ntiles, general_theta, fp8):
    import concourse.bass as bass
    import concourse.tile as tile
    from concourse import bacc, mybir

    f32 = mybir.dt.float32
    bf16 = mybir.dt.bfloat16
    fp8e4 = mybir.dt.float8e4
    edt = fp8e4 if fp8 else bf16
    AF = mybir.ActivationFunctionType
    OP = mybir.AluOpType
    DR = mybir.MatmulPerfMode.DoubleRow
    nshard = ntiles * _P
    G = _G
    ng = ntiles // G
    assert ntiles % G == 0

    CDEN = 1.0 + _K * _K * _RHO
    CONST = (_K * _RHO) / CDEN

    nc = bacc.Bacc("TRN2", target_bir_lowering=False, debug=False)
    epsT_d = nc.declare_dram_parameter("epsT", [ng, _P, G, _KC, _P], edt, isOutput=False)
    C_d = nc.declare_dram_parameter("Cmat", [_P, _KC, _K], edt, isOutput=False)
    c0_d = nc.declare_dram_parameter("c0rows", [3, _K], bf16, isOutput=False)
    if general_theta:
        invth_d = nc.declare_dram_parameter("invtheta", [_P, _K], f32, isOutput=False)
    gamma_d = nc.declare_dram_parameter("gamma", [nshard, _K], f32, isOutput=True)
    # partition d owns rows [g*1024 + d*8 .. +8): per-partition-contiguous
    # 16KB runs in the row-major output (few, large DMA descriptors)
    gv = gamma_d[:].rearrange("(ng d t) k -> ng d t k", d=_P, t=G)

    with tile.TileContext(nc) as tc:
        with (
            tc.tile_pool(name="const", bufs=1) as constp,
            tc.tile_pool(name="eps", bufs=4) as epsp,
            tc.tile_pool(name="psum", bufs=4, space=bass.MemorySpace.PSUM) as psump,
            tc.tile_pool(name="work", bufs=2 * _G) as workp,
            tc.tile_pool(name="gout", bufs=3) as goutp,
            tc.tile_pool(name="stat", bufs=3) as statp,
        ):
            Ct = constp.tile([_P, _KC, _K], edt)
            nc.gpsimd.dma_start(Ct[:], C_d[:])
            c0t = constp.tile([3, _K], bf16)
            nc.gpsimd.dma_start(c0t[:], c0_d[:])
            ones3 = constp.tile([3, _P], bf16)
            nc.vector.memset(ones3[:], 1.0)
            if general_theta:
                invtht = constp.tile([_P, _K], f32)
                nc.gpsimd.dma_start(invtht[:], invth_d[:])

            inv_scale = 1.0  # set per build for fp8 via module global below
            if fp8:
                inv_scale = _fp8_inv_scale[0]

            for gi in range(ng):
                egt = epsp.tile([_P, G, _KC, _P], edt, tag="eps")
                nc.gpsimd.dma_start(egt[:], epsT_d[gi])
                gbuf = goutp.tile([_P, G, _K], f32, tag="gbuf")

                es = []
                Tg = statp.tile([_P, G], f32, tag="T")
                Vg = statp.tile([_P, G], f32, tag="V") if general_theta else None
                for t in range(G):
                    ps = psump.tile([_P, _K], f32, tag="ps")
                    # bias first: resets PSUM to the sum of 3 bias rows
                    nc.tensor.matmul(ps[:], ones3[:], c0t[:], start=True, stop=False)
                    if fp8:
                        for c in (0, 2):
                            nc.tensor.matmul(
                                ps[:], egt[:, t, c:c + 2, :], Ct[:, c:c + 2, :],
                                start=False, stop=(c == 2), perf_mode=DR,
                            )
                    else:
                        for c in range(_KC):
                            nc.tensor.matmul(
                                ps[:], egt[:, t, c, :], Ct[:, c, :],
                                start=False, stop=(c == _KC - 1),
                            )

                    e = workp.tile([_P, _K], f32, tag="e")
                    nc.scalar.activation(e[:], ps[:], AF.Exp,
                                         scale=inv_scale,
                                         accum_out=Tg[:, t:t + 1])
                    es.append(e)
                    if general_theta:
                        scratch = workp.tile([_P, _K], f32, tag="scratch")
                        nc.vector.tensor_tensor_reduce(
                            out=scratch[:], in0=e[:], in1=invtht[:],
                            scale=1.0, scalar=0.0,
                            op0=OP.mult, op1=OP.add,
                            accum_out=Vg[:, t:t + 1],
                        )

                # group-batched stats: one mul + one reciprocal per 8 tiles
                s1g = statp.tile([_P, G], f32, tag="s1")
                if not general_theta:
                    dtmp = statp.tile([_P, G], f32, tag="dtmp")
                    nc.vector.tensor_scalar_mul(dtmp[:], Tg[:], CDEN)
                    nc.vector.reciprocal(s1g[:], dtmp[:])
                    for t in range(G):
                        nc.vector.tensor_scalar(
                            gbuf[:, t, :], es[t][:], s1g[:, t:t + 1], CONST,
                            OP.mult, OP.add,
                        )
                else:
                    # D = K*RHO*V + T;  s1 = 1/D;  s0 = RHO*V*s1
                    Dg = statp.tile([_P, G], f32, tag="D")
                    nc.vector.tensor_scalar_mul(Dg[:], Vg[:], float(_K * _RHO))
                    nc.vector.tensor_add(Dg[:], Dg[:], Tg[:])
                    nc.vector.reciprocal(s1g[:], Dg[:])
                    s0g = statp.tile([_P, G], f32, tag="s0")
                    nc.vector.tensor_mul(s0g[:], Vg[:], s1g[:])
                    nc.vector.tensor_scalar_mul(s0g[:], s0g[:], _RHO)
                    for t in range(G):
                        nc.vector.tensor_scalar(
                            gbuf[:, t, :], es[t][:], s1g[:, t:t + 1],
                            s0g[:, t:t + 1], OP.mult, OP.add,
                        )

                nc.gpsimd.dma_start(gv[gi], gbuf[:])
    nc.compile()
    return nc


_fp8_inv_scale = [1.0]  # exp-input rescale for the fp8 build (2**-s)


def _softmax_rows(x):
    m = x.max(axis=-1, keepdims=True)
    e = np.exp(x - m)
    return e / e.sum(axis=-1, keepdims=True)


def _host_prep(alpha, beta, sigma, mu, eps):
    """Fold the small parameters; shard + transpose/cast eps."""
    theta = _softmax_rows(alpha.astype(np.float64))            # [K]
    B = _softmax_rows(beta.astype(np.float64))                 # [K, K]
    L = np.linalg.cholesky(sigma.astype(np.float64))           # [K, K]
    C = L.T @ B                                                # [K, K]
    c0 = mu.astype(np.float64) @ B + np.log(theta)             # [K]

    uniform = bool(np.max(np.abs(theta - 1.0 / _K)) < 1e-12)

    # fp8 pre-scale: put max|C|*2^s around ~96 (TRN e4m3 max normal is 240)
    scale_log2 = 0
    if _FP8:
        maxc = float(np.abs(C).max())
        scale_log2 = int(np.floor(np.log2(96.0 / maxc))) if maxc > 0 else 0
    scl = float(2.0 ** scale_log2)

    # C chunk layout [P, KC, K]: element [p, c, j] = C[c*P + p, j]
    Cb = np.ascontiguousarray(
        (C * scl).reshape(_KC, _P, _K).transpose(1, 0, 2)
    ).astype(_EDT())

    # 3-row hi/mid/lo bf16 split of c0*scl so the bias is fp32-exact on device
    c0f = (c0 * scl).astype(np.float32)
    r0 = c0f.astype(_BF16)
    r1 = (c0f - r0.astype(np.float32)).astype(_BF16)
    r2 = (c0f - r0.astype(np.float32) - r1.astype(np.float32)).astype(_BF16)
    c0rows = np.stack([r0, r1, r2], axis=0)                    # [3, K] bf16

    invtheta = None
    if not uniform:
        invtheta = np.broadcast_to(
            (1.0 / theta).astype(np.float32), (_P, _K)
        ).copy()

    shards = [
        _prep_eps_shard(eps[core * _NSHARD:(core + 1) * _NSHARD])
        for core in range(_NCORES)
    ]
    return Cb, c0rows, invtheta, uniform, scale_log2, shards


def _EDT():
    return ml_dtypes.float8_e4m3 if _FP8 else _BF16


def _prep_eps_shard(sh):
    """[rows, K] -> [ng, P(k-sub), G(tile), KC, P(doc-lane)].

    Row assignment: lane d of sub-tile t in group g covers row
    g*1024 + d*8 + t, so each partition's group output is 8 consecutive
    rows (one contiguous 16KB DMA run)."""
    ntiles = sh.shape[0] // _P
    ng = ntiles // _G
    sh5 = sh.reshape(ng, _P, _G, _KC, _P)                 # [g, d, t, c, p]
    return np.ascontiguousarray(sh5.transpose(0, 4, 2, 3, 1)).astype(_EDT())


def kernel(bow, alpha, beta, sigma, mu, eps):
    from concourse.bass_utils import run_bass_kernel_spmd

    Cb, c0rows, invtheta, uniform, scale_log2, shards = _host_prep(
        alpha, beta, sigma, mu, eps)

    key = (_NTILES, not uniform, _FP8, scale_log2)
    if key not in _prog_cache:
        _fp8_inv_scale[0] = float(2.0 ** -scale_log2)
        _prog_cache[key] = _build_program(_NTILES, not uniform, _FP8)
    nc = _prog_cache[key]

    in_maps = []
    for core in range(_NCORES):
        m = {"epsT": shards[core], "Cmat": Cb, "c0rows": c0rows}
        if not uniform:
            m["invtheta"] = invtheta
        in_maps.append(m)

    global _last_results
    res = run_bass_kernel_spmd(nc, in_maps, list(range(_NCORES)), trace=_trace)
    _last_results = res
    out = np.concatenate([res.results[i]["gamma"] for i in range(_NCORES)], axis=0)
    return np.ascontiguousarray(out.astype(np.float32))
